# revision 33
# baseline (speedup 1.0000x reference)
"""ContourLoss on 8 Trainium2 NeuronCores (data parallel over batch B=8).

Device work per core (one sample):
  - Intersection grid over compacted valid points: for segment pairs i,j
        d1*d2 = <V(i), U(j)>,  d3*d4 = <U(i), V(j)>
    with U/V 6 quadratic per-segment features -> two small-K matmuls per
    [128 x N] tile on the tensor engine.  Per-sample validity (i,j < n_seg)
    is folded into two extra feature rows adding +BIG to q12 so
    sigmoid(-0.01*q12) underflows to exactly 0.
  - sigmoid(-0.01*q) on the scalar engine (scale fused), product on the
    vector engine, per-tile row-sum into a private column of a [128, 64]
    partials tile (no cross-tile serialization), host sums columns.
  - Triangular mask (j >= i+2) only affects the leading 132-wide chunk of
    each 128-row band; handled there with a constant 0/1 tile.
  - CE / SmoothL1 / cosine terms are small [128, <=64] elementwise work.
The host does the O(B*S) prep (compaction order, masks, features) and the
final scalar arithmetic; denominators/counts and the reference's excluded
wrap pair (i=0, j=n_seg-1) are computed host-side.
"""

import numpy as np

RETINA = 224.0
NUM_CLASSES = 4
B = 8
S = 2048
M = S - 1
NCORES = 8
W_DIAG = 132  # leading chunk per 128-band: covers all cells with j-i < 2
BIG = 1.0e13
NCOLS = 64    # partials tile width: 5 cheap cols + per-block isect cols
ISECT_COL0 = 5

_CACHE = {}
TRACE_KWARGS = {}  # test harness sets e.g. {"trace": True} to profile
LAST_RESULTS = None
SIG_BF16 = True   # sigmoid outputs + products in bf16 (2x DVE modes)
ACT_4BANK = True  # single ACT op over a 4-bank PSUM pair tile
INPLACE_MASK = True  # apply tri mask in place on s12
# The q matmuls run in bf16 at full PE rate, with fp32-grade precision via a
# 3-way bf16 split of each feature stacked along K (K is free on the PE):
#   x = hi + mid + lo;  <A,B> = sum of group products
#   groups: (Ah,Bh) (Am,Bh) (Ah,Bm) (Al,Bh) (Ah,Bl) (Am,Bm)   [mid*lo dropped]
KQ12 = 6 * 8   # 48 rows
KQ34 = 6 * 6   # 36 rows


# ---------------------------------------------------------------------------
# walrus in this environment accepts at most ONE sync-wait per instruction;
# the pinned concourse Tile stack can attach several (notably the kernel-tail
# Drain).  Splitting extras onto same-engine NoOps is semantically identical.
def _split_multi_waits(nc, max_waits=1):
    import concourse.mybir as mybir
    n_split = 0
    for fn in nc.m.functions:
        for blk in fn.blocks:
            out = []
            changed = False
            for inst in blk.instructions:
                si = inst.sync_info
                ow = list(si.on_wait) if (si is not None and si.on_wait) else []
                if len(ow) > max_waits:
                    for k, w in enumerate(ow[:-max_waits]):
                        out.append(mybir.InstNoOp(
                            name=f"{inst.name}_wsplit{k}",
                            engine=inst.engine,
                            ins=[], outs=[],
                            sync_info=mybir.SyncInfo(on_wait=[w],
                                                     on_update=[]),
                        ))
                        n_split += 1
                    si.on_wait = ow[-max_waits:]
                    changed = True
                out.append(inst)
            if changed:
                blk.instructions = out
    return n_split


def _schedule(L, Jmax):
    """Superblocks of 1-2 chunks.  Chunk = (i0, j0, N, first); `first`
    chunks carry the triangular mask on their leading W_DIAG columns.
    512-wide chunks are paired so ACT/DVE ops cover 2 chunks at once."""
    chunks = []
    for ib in range(L // 128):
        i0 = 128 * ib
        j0 = i0
        first = True
        while j0 < Jmax:
            N = min(512, Jmax - j0)
            chunks.append((i0, j0, N, first))
            j0 += N
            first = False
    full = [c for c in chunks if c[2] == 512]
    tails = [c for c in chunks if c[2] != 512]
    sbs = [full[i:i + 2] for i in range(0, len(full), 2)]
    sbs += [[t] for t in tails]
    return sbs


def _host_prep(pp, op, cp, ts, pm):
    """Per-sample compaction + feature construction (all O(B*S))."""
    tc_cls = ts[:, :, 4].astype(np.int32)
    tp = ts[:, :, :2]
    to = ts[:, :, 2:4]
    valid = ~pm
    nn = valid & (tc_cls != 0)

    per_core = []
    n_segs = []
    for b in range(B):
        order = np.argsort(~nn[b], kind="stable")
        pts = pp[b][order].astype(np.float64)
        n = int(nn[b].sum())
        n_seg = n - 1
        n_segs.append(n_seg)
        if n > 0:
            pts = pts - pts[:n].mean(axis=0)
        sx, sy = pts[:-1, 0], pts[:-1, 1]
        eX, eY = pts[1:, 0], pts[1:, 1]
        ex, ey = eX - sx, eY - sy
        c = ex * sy - ey * sx
        g0, g1, g2 = ex, -ey, -c
        one = np.ones(M)
        # f1 = (sy, sx, 1), f2 = (eY, eX, 1)
        U6 = np.stack([g0 * g0, g1 * g1, g2 * g2,
                       g0 * g1, g0 * g2, g1 * g2], 0)
        V6 = np.stack([sy * eY, sx * eX, one,
                       sy * eX + sx * eY,
                       sy + eY,
                       sx + eX], 0)
        inv = (np.arange(M) >= max(n_seg, 0)).astype(np.float64) * BIG
        fA12 = np.concatenate([V6, inv[None], one[None]], 0)  # [8, M]
        fB12 = np.concatenate([U6, one[None], inv[None]], 0)  # [8, M]
        per_core.append(dict(n=n, n_seg=n_seg,
                             A12=_split_stack_A(fA12), B12=_split_stack_B(fB12),
                             A34=_split_stack_A(U6), B34=_split_stack_B(V6)))
    return tc_cls, tp, to, valid, nn, per_core, n_segs


def _split3(x):
    import ml_dtypes
    bf = ml_dtypes.bfloat16
    hi = x.astype(bf).astype(np.float64)
    mid = (x - hi).astype(bf).astype(np.float64)
    lo = (x - hi - mid).astype(bf).astype(np.float64)
    return hi, mid, lo


def _split_stack_A(A):
    h, m, l = _split3(A)
    import ml_dtypes
    return np.concatenate([h, m, h, l, h, m], 0).astype(ml_dtypes.bfloat16)


def _split_stack_B(Bm):
    h, m, l = _split3(Bm)
    import ml_dtypes
    return np.concatenate([h, h, m, h, l, m], 0).astype(ml_dtypes.bfloat16)


def _build_program(L, Jmax):
    import concourse.bass as bass
    import concourse.tile as tile
    from concourse import mybir

    f32 = mybir.dt.float32
    ALU = mybir.AluOpType
    ACT = mybir.ActivationFunctionType
    AX = mybir.AxisListType

    sched = _schedule(L, Jmax)
    assert ISECT_COL0 + len(sched) <= NCOLS, (L, len(sched))

    bf16 = mybir.dt.bfloat16
    tri_dt = bf16 if SIG_BF16 else f32
    nc = bass.Bass()
    # packed split features (bf16):
    # [48, 4*L] = A12 | B12 | A34(rows 0:36) | B34(rows 0:36)
    d_feat = nc.dram_tensor("feat", [KQ12, 4 * L], bf16, kind="ExternalInput")
    d_tri = nc.dram_tensor("tri", [128, W_DIAG], tri_dt, kind="ExternalInput")
    # packed per-token data: ppn|tpn|opr|ton|cp4|ohv|vf
    d_tok = nc.dram_tensor("tok", [128, 272], f32, kind="ExternalInput")
    d_out = nc.dram_tensor("partials", [128, NCOLS], f32,
                           kind="ExternalOutput")

    with tile.TileContext(nc) as tc:
        with (
            tc.tile_pool(name="singles", bufs=1) as singles,
            tc.tile_pool(name="sig", bufs=3) as sig,
            tc.tile_pool(name="psum", bufs=2, space="PSUM") as psum,
        ):
            feat = singles.tile([KQ12, 4 * L], bf16)
            tri = singles.tile([128, W_DIAG], tri_dt)
            tok = singles.tile([128, 272], f32)
            nc.sync.dma_start(out=feat[:], in_=d_feat[:])
            nc.sync.dma_start(out=tok[:], in_=d_tok[:])
            nc.sync.dma_start(out=tri[:], in_=d_tri[:])

            fA12 = feat[:, 0 * L:1 * L]
            fB12 = feat[:, 1 * L:2 * L]
            fA34 = feat[0:KQ34, 2 * L:3 * L]
            fB34 = feat[0:KQ34, 3 * L:4 * L]
            ppn = tok[:, 0:32]
            tpn = tok[:, 32:64]
            opr = tok[:, 64:96]
            ton = tok[:, 96:128]
            cp4 = tok[:, 128:192]
            ohv = tok[:, 192:256]
            vf = tok[:, 256:272]

            cols = singles.tile([128, NCOLS], f32)
            junk = singles.tile([128, 512], f32)
            dpt = singles.tile([128, 32], f32)
            e4 = singles.tile([128, 64], f32)
            gs = singles.tile([128, 16], f32)
            lg = singles.tile([128, 16], f32)

            nc.vector.memset(cols, 0.0)

            # ---- cheap losses (ACT: Exp then Ln, before the sigmoid set) ---
            # col0: sum (pp-tp)^2 * nn   (host scales by 0.25/RET^2)
            nc.vector.tensor_tensor(out=dpt[:], in0=ppn, in1=tpn,
                                    op=ALU.subtract)
            nc.vector.tensor_tensor(out=junk[:, :32], in0=dpt[:], in1=dpt[:],
                                    op=ALU.mult)
            nc.vector.tensor_reduce(out=cols[:, 0:1], in_=junk[:, :32],
                                    axis=AX.X, op=ALU.add)
            # col1: sum (op . to) * nn
            nc.vector.tensor_tensor(out=junk[:, :32], in0=opr, in1=ton,
                                    op=ALU.mult)
            nc.vector.tensor_reduce(out=cols[:, 1:2], in_=junk[:, :32],
                                    axis=AX.X, op=ALU.add)
            # col2: sum lse*vf ; col3: sum x_sel*vf
            nc.scalar.activation(out=e4[:], in_=cp4, func=ACT.Exp)
            nc.vector.tensor_reduce(
                out=gs[:], in_=e4[:].rearrange("p (t c) -> p t c", c=4),
                axis=AX.X, op=ALU.add)
            nc.scalar.activation(out=lg[:], in_=gs[:], func=ACT.Ln)
            nc.vector.tensor_tensor(out=junk[:, :16], in0=lg[:], in1=vf,
                                    op=ALU.mult)
            nc.vector.tensor_reduce(out=cols[:, 2:3], in_=junk[:, :16],
                                    axis=AX.X, op=ALU.add)
            nc.vector.tensor_tensor(out=junk[:, :64], in0=cp4, in1=ohv,
                                    op=ALU.mult)
            nc.vector.tensor_reduce(out=cols[:, 3:4], in_=junk[:, :64],
                                    axis=AX.X, op=ALU.add)

            # ---- intersection grid ----
            sdt = bf16 if SIG_BF16 else f32

            for blk, sb in enumerate(sched):
                # per chunk k: q12 -> bank 2k, q34 -> bank 2k+1
                q = psum.tile([128, 2048], f32, tag="q")
                for k, (i0, j0, N, first) in enumerate(sb):
                    nc.tensor.matmul(q[:, 1024 * k:1024 * k + N],
                                     fA12[:, i0:i0 + 128],
                                     fB12[:, j0:j0 + N],
                                     start=True, stop=True)
                    nc.tensor.matmul(q[:, 1024 * k + 512:1024 * k + 512 + N],
                                     fA34[:, i0:i0 + 128],
                                     fB34[:, j0:j0 + N],
                                     start=True, stop=True)
                s12 = sig.tile([128, 2048], sdt, tag="s12")
                if len(sb) == 2 and ACT_4BANK:  # two full 512-chunk pairs
                    nc.scalar.activation(out=s12[:], in_=q[:],
                                         func=ACT.Sigmoid, scale=-0.01)
                elif len(sb) == 2:
                    nc.scalar.activation(out=s12[:, 0:1024], in_=q[:, 0:1024],
                                         func=ACT.Sigmoid, scale=-0.01)
                    nc.scalar.activation(out=s12[:, 1024:2048],
                                         in_=q[:, 1024:2048],
                                         func=ACT.Sigmoid, scale=-0.01)
                else:
                    N = sb[0][2]
                    qv = q[:, 0:1024].rearrange(
                        "p (two n) -> p two n", two=2)[:, :, :N]
                    sv = s12[:, 0:1024].rearrange(
                        "p (two n) -> p two n", two=2)[:, :, :N]
                    nc.scalar.activation(out=sv, in_=qv,
                                         func=ACT.Sigmoid, scale=-0.01)
                # triangular mask on the s1 quadrant
                for k, (i0, j0, N, first) in enumerate(sb):
                    if first:
                        W = min(W_DIAG, N)
                        off = 1024 * k
                        if INPLACE_MASK:
                            nc.vector.tensor_tensor(
                                out=s12[:, off:off + W],
                                in0=s12[:, off:off + W],
                                in1=tri[:, :W], op=ALU.mult)
                        else:
                            s1m = sig.tile([128, W_DIAG], sdt, tag="s1m")
                            nc.vector.tensor_tensor(
                                out=s1m[:, :W], in0=s12[:, off:off + W],
                                in1=tri[:, :W], op=ALU.mult)
                            nc.vector.tensor_copy(
                                out=s12[:, off:off + W], in_=s1m[:, :W])
                # s1*s2 product, then row-sum into this superblock's column
                prod = sig.tile([128, 1024], sdt, tag="prod")
                if len(sb) == 2:
                    v = s12[:].rearrange("p (k n) -> p k n", n=512)
                    nc.vector.tensor_tensor(
                        out=prod[:].rearrange("p (k n) -> p k n", n=512),
                        in0=v[:, 0::2, :], in1=v[:, 1::2, :], op=ALU.mult)
                    red_in = prod[:]
                else:
                    N = sb[0][2]
                    nc.vector.tensor_tensor(out=prod[:, :N],
                                            in0=s12[:, 0:N],
                                            in1=s12[:, 512:512 + N],
                                            op=ALU.mult)
                    red_in = prod[:, :N]
                c = ISECT_COL0 + blk
                nc.vector.tensor_reduce(out=cols[:, c:c + 1], in_=red_in,
                                        axis=AX.X, op=ALU.add)

            nc.sync.dma_start(out=d_out[:], in_=cols[:])

    _split_multi_waits(nc)
    return nc


def kernel(point_pred, orient_pred, class_pred, target_seq, padding_mask):
    pp = np.ascontiguousarray(np.asarray(point_pred, dtype=np.float32))
    op = np.ascontiguousarray(np.asarray(orient_pred, dtype=np.float32))
    cp = np.ascontiguousarray(np.asarray(class_pred, dtype=np.float32))
    ts = np.ascontiguousarray(np.asarray(target_seq, dtype=np.float32))
    pm = np.ascontiguousarray(np.asarray(padding_mask)).astype(bool)

    tc_cls, tp, to, valid, nn, per_core, n_segs = _host_prep(pp, op, cp, ts, pm)

    # grid bounds: i-bands padded to 128; j truncated at max n_seg (mult of 4)
    nsmax = max(max(n_segs), 1)
    L = max(128, -(-nsmax // 128) * 128)
    L = min(L, -(-M // 128) * 128)  # ceil(M/128)*128 = 2048 max
    Jmax = min(-(-nsmax // 4) * 4, L)

    key = (L, Jmax, SIG_BF16, ACT_4BANK, INPLACE_MASK)
    if key not in _CACHE:
        _CACHE[key] = _build_program(L, Jmax)
    nc = _CACHE[key]

    # triangular mask for the leading chunk of each band: keep j-i >= 2
    ii = np.arange(128)[:, None]
    jj = np.arange(W_DIAG)[None, :]
    tri = (jj >= ii + 2).astype(np.float32)
    if SIG_BF16:
        import ml_dtypes
        tri = tri.astype(ml_dtypes.bfloat16)

    import ml_dtypes
    bfdt = ml_dtypes.bfloat16
    eye = np.eye(NUM_CLASSES, dtype=np.float32)
    in_maps = []
    for b in range(B):
        pc = per_core[b]
        featpk = np.zeros((KQ12, 4 * L), bfdt)
        w = min(M, L)
        featpk[:KQ12, 0 * L:0 * L + w] = pc["A12"][:, :w]
        featpk[:KQ12, 1 * L:1 * L + w] = pc["B12"][:, :w]
        featpk[:KQ34, 2 * L:2 * L + w] = pc["A34"][:, :w]
        featpk[:KQ34, 3 * L:3 * L + w] = pc["B34"][:, :w]
        if L > M:
            # mask the padded tail: A12 row6=BIG*inv_i / row7=ones pattern.
            # In the split stacking, group g occupies rows 8g..8g+7 with
            # A-pattern [h,m,h,l,h,m] / B-pattern [h,h,m,h,l,m]; setting the
            # hi part of group 0 is enough (BIG in bf16 is still huge).
            big_bf = bfdt(BIG)
            featpk[6, 0 * L + M:1 * L] = big_bf   # A12 g0(hi) row6 = BIG
            featpk[7, 0 * L + M:1 * L] = bfdt(1.0)
            featpk[6, 1 * L + M:2 * L] = bfdt(1.0)
            featpk[7, 1 * L + M:2 * L] = big_bf

        nf = nn[b].astype(np.float32)[:, None]
        vfb = valid[b].astype(np.float32)
        tokpk = np.concatenate([
            (pp[b] * nf).reshape(128, 32),
            (tp[b] * nf).reshape(128, 32),
            op[b].reshape(128, 32),
            (to[b] * nf).reshape(128, 32),
            cp[b].reshape(128, 64),
            (eye[tc_cls[b]] * vfb[:, None]).reshape(128, 64),
            vfb.reshape(128, 16),
        ], axis=1).astype(np.float32)
        in_maps.append({
            "feat": np.ascontiguousarray(featpk),
            "tri": tri,
            "tok": np.ascontiguousarray(tokpk),
        })

    from concourse.bass_utils import run_bass_kernel_spmd
    global LAST_RESULTS
    kw = dict(TRACE_KWARGS) if TRACE_KWARGS else {}
    res = run_bass_kernel_spmd(nc, in_maps, core_ids=list(range(NCORES)), **kw)
    LAST_RESULTS = res
    parts = [r["partials"] for r in res.results]  # each [128, NCOLS] f32

    f32 = np.float32
    pt_raw = f32(0); cos_sum = f32(0); lse_sum = f32(0); sel_sum = f32(0)
    isect_sum = f32(0)
    for b in range(B):
        p = parts[b].astype(np.float32)
        pt_raw += p[:, 0].sum(dtype=np.float32)
        cos_sum += p[:, 1].sum(dtype=np.float32)
        lse_sum += p[:, 2].sum(dtype=np.float32)
        sel_sum += p[:, 3].sum(dtype=np.float32)
        isect_sum += p[:, ISECT_COL0:].sum(dtype=np.float32)

    # wrap-pair exclusion + pair count (host, exact)
    wrap_sum = np.float64(0.0)
    cnt_total = 0
    for b in range(B):
        pc = per_core[b]
        n, n_seg = pc["n"], pc["n_seg"]
        if n < 4:
            continue
        cnt_total += (n_seg - 1) * (n_seg - 2) // 2 - 1
        jw = n_seg - 1
        q12w = np.float32(
            np.dot(pc["A12"][:, 0].astype(np.float32),
                   pc["B12"][:, jw].astype(np.float32)))
        q34w = np.float32(
            np.dot(pc["A34"][:, 0].astype(np.float32),
                   pc["B34"][:, jw].astype(np.float32)))
        with np.errstate(over="ignore"):
            sw = (1.0 / (1.0 + np.exp(np.float64(0.01) * q12w)) *
                  1.0 / (1.0 + np.exp(np.float64(0.01) * q34w)))
        wrap_sum += sw

    valid_cnt = f32(valid.sum())
    nn_cnt = f32(nn.sum())
    vden = max(valid_cnt, f32(1.0))
    nden = max(nn_cnt, f32(1.0))

    pt_loss = f32(pt_raw * f32(0.25 / (RETINA * RETINA)) / nden)
    orient_loss = f32((nn_cnt - cos_sum) / nden)
    cls_loss = f32((lse_sum - sel_sum) / vden)
    if cnt_total > 0:
        isect_loss = f32((np.float64(isect_sum) - wrap_sum) / cnt_total)
    else:
        isect_loss = f32(0.0)
    total = f32(pt_loss + f32(0.5) * orient_loss + cls_loss
                + f32(0.1) * isect_loss)
    return (total, pt_loss, orient_loss, cls_loss, isect_loss)


# revision 40
# speedup vs baseline: 1.0367x; 1.0367x over previous
"""ContourLoss on 8 Trainium2 NeuronCores (data parallel over batch B=8).

Device work per core (one sample):
  - Intersection grid over compacted valid points: for segment pairs i,j
        d1*d2 = <V(i), U(j)>,  d3*d4 = <U(i), V(j)>
    with U/V 6 quadratic per-segment features -> two small-K matmuls per
    [128 x N] tile on the tensor engine.  Per-sample validity (i,j < n_seg)
    is folded into two extra feature rows adding +BIG to q12 so
    sigmoid(-0.01*q12) underflows to exactly 0.
  - sigmoid(-0.01*q) on the scalar engine (scale fused), product on the
    vector engine, per-tile row-sum into a private column of a [128, 64]
    partials tile (no cross-tile serialization), host sums columns.
  - Triangular mask (j >= i+2) only affects the leading 132-wide chunk of
    each 128-row band; handled there with a constant 0/1 tile.
  - CE / SmoothL1 / cosine terms are small [128, <=64] elementwise work.
The host does the O(B*S) prep (compaction order, masks, features) and the
final scalar arithmetic; denominators/counts and the reference's excluded
wrap pair (i=0, j=n_seg-1) are computed host-side.
"""

import numpy as np

RETINA = 224.0
NUM_CLASSES = 4
B = 8
S = 2048
M = S - 1
NCORES = 8
W_DIAG = 132  # leading chunk per 128-band: covers all cells with j-i < 2
BIG = 1.0e13
NCOLS = 64    # partials tile width: 5 cheap cols + per-block isect cols
ISECT_COL0 = 5

_CACHE = {}
TRACE_KWARGS = {}  # test harness sets e.g. {"trace": True} to profile
LAST_RESULTS = None
SIG_BF16 = True   # sigmoid outputs + products in bf16 (2x DVE modes)
ACT_4BANK = False  # a single ACT op reading 4 PSUM banks crashes the device
INPLACE_MASK = True  # apply tri mask in place on s12
GP_PROD = False  # GpSimd tensor ops crash the device on this walrus/HW
# The q matmuls run in bf16 at full PE rate, with fp32-grade precision via a
# 3-way bf16 split of each feature stacked along K (K is free on the PE):
#   x = hi + mid + lo;  <A,B> = sum of group products
#   groups: (Ah,Bh) (Am,Bh) (Ah,Bm) (Al,Bh) (Ah,Bl) (Am,Bm)   [mid*lo dropped]
KQ12 = 6 * 8   # 48 rows
KQ34 = 6 * 6   # 36 rows


# ---------------------------------------------------------------------------
# walrus in this environment accepts at most ONE sync-wait per instruction;
# the pinned concourse Tile stack can attach several (notably the kernel-tail
# Drain).  Splitting extras onto same-engine NoOps is semantically identical.
def _split_multi_waits(nc, max_waits=1):
    import concourse.mybir as mybir
    n_split = 0
    for fn in nc.m.functions:
        for blk in fn.blocks:
            out = []
            changed = False
            for inst in blk.instructions:
                si = inst.sync_info
                ow = list(si.on_wait) if (si is not None and si.on_wait) else []
                if len(ow) > max_waits:
                    for k, w in enumerate(ow[:-max_waits]):
                        out.append(mybir.InstNoOp(
                            name=f"{inst.name}_wsplit{k}",
                            engine=inst.engine,
                            ins=[], outs=[],
                            sync_info=mybir.SyncInfo(on_wait=[w],
                                                     on_update=[]),
                        ))
                        n_split += 1
                    si.on_wait = ow[-max_waits:]
                    changed = True
                out.append(inst)
            if changed:
                blk.instructions = out
    return n_split


def _schedule(L, Jmax):
    """Superblocks of 1-2 chunks.  Chunk = (i0, j0, N, first); `first`
    chunks carry the triangular mask on their leading W_DIAG columns.
    512-wide chunks are paired so ACT/DVE ops cover 2 chunks at once."""
    chunks = []
    for ib in range(L // 128):
        i0 = 128 * ib
        j0 = i0
        first = True
        while j0 < Jmax:
            N = min(512, Jmax - j0)
            chunks.append((i0, j0, N, first))
            j0 += N
            first = False
    full = [c for c in chunks if c[2] == 512]
    tails = [c for c in chunks if c[2] != 512]
    # first block alone and with minimal j-extent so compute starts as soon
    # as the first slice of the feature DMA lands
    head = [[full[0]]] if full else []
    rest = full[1:]
    sbs = head + [rest[i:i + 2] for i in range(0, len(rest), 2)]
    sbs += [[t] for t in tails]
    return sbs


def _host_prep(pp, op, cp, ts, pm):
    """Per-sample compaction + feature construction (all O(B*S))."""
    tc_cls = ts[:, :, 4].astype(np.int32)
    tp = ts[:, :, :2]
    to = ts[:, :, 2:4]
    valid = ~pm
    nn = valid & (tc_cls != 0)

    per_core = []
    n_segs = []
    for b in range(B):
        order = np.argsort(~nn[b], kind="stable")
        pts = pp[b][order].astype(np.float64)
        n = int(nn[b].sum())
        n_seg = n - 1
        n_segs.append(n_seg)
        if n > 0:
            pts = pts - pts[:n].mean(axis=0)
        sx, sy = pts[:-1, 0], pts[:-1, 1]
        eX, eY = pts[1:, 0], pts[1:, 1]
        ex, ey = eX - sx, eY - sy
        c = ex * sy - ey * sx
        g0, g1, g2 = ex, -ey, -c
        one = np.ones(M)
        # f1 = (sy, sx, 1), f2 = (eY, eX, 1)
        U6 = np.stack([g0 * g0, g1 * g1, g2 * g2,
                       g0 * g1, g0 * g2, g1 * g2], 0)
        V6 = np.stack([sy * eY, sx * eX, one,
                       sy * eX + sx * eY,
                       sy + eY,
                       sx + eX], 0)
        inv = (np.arange(M) >= max(n_seg, 0)).astype(np.float64) * BIG
        fA12 = np.concatenate([V6, inv[None], one[None]], 0)  # [8, M]
        fB12 = np.concatenate([U6, one[None], inv[None]], 0)  # [8, M]
        per_core.append(dict(n=n, n_seg=n_seg,
                             A12=_split_stack_A(fA12), B12=_split_stack_B(fB12),
                             A34=_split_stack_A(U6), B34=_split_stack_B(V6)))
    return tc_cls, tp, to, valid, nn, per_core, n_segs


def _split3(x):
    import ml_dtypes
    bf = ml_dtypes.bfloat16
    hi = x.astype(bf).astype(np.float64)
    mid = (x - hi).astype(bf).astype(np.float64)
    lo = (x - hi - mid).astype(bf).astype(np.float64)
    return hi, mid, lo


def _split_stack_A(A):
    h, m, l = _split3(A)
    import ml_dtypes
    return np.concatenate([h, m, h, l, h, m], 0).astype(ml_dtypes.bfloat16)


def _split_stack_B(Bm):
    h, m, l = _split3(Bm)
    import ml_dtypes
    return np.concatenate([h, h, m, h, l, m], 0).astype(ml_dtypes.bfloat16)


def _build_program(L, Jmax):
    import concourse.bass as bass
    import concourse.tile as tile
    from concourse import mybir

    f32 = mybir.dt.float32
    ALU = mybir.AluOpType
    ACT = mybir.ActivationFunctionType
    AX = mybir.AxisListType

    sched = _schedule(L, Jmax)
    assert ISECT_COL0 + len(sched) <= NCOLS, (L, len(sched))

    bf16 = mybir.dt.bfloat16
    tri_dt = bf16 if SIG_BF16 else f32
    nc = bass.Bass()
    # packed split features (bf16):
    # [48, 4*L] = A12 | B12 | A34(rows 0:36) | B34(rows 0:36)
    d_feat = nc.dram_tensor("feat", [KQ12, 4 * L], bf16, kind="ExternalInput")
    d_tri = nc.dram_tensor("tri", [128, W_DIAG], tri_dt, kind="ExternalInput")
    # packed per-token data: ppn|tpn|opr|ton|cp4|ohv|vf
    d_tok = nc.dram_tensor("tok", [128, 272], f32, kind="ExternalInput")
    d_out = nc.dram_tensor("partials", [128, NCOLS], f32,
                           kind="ExternalOutput")

    with tile.TileContext(nc) as tc:
        with (
            tc.tile_pool(name="singles", bufs=1) as singles,
            tc.tile_pool(name="sig", bufs=3) as sig,
            tc.tile_pool(name="psum", bufs=2, space="PSUM") as psum,
        ):
            feat = singles.tile([KQ12, 4 * L], bf16)
            tri = singles.tile([128, W_DIAG], tri_dt)
            tok = singles.tile([128, 272], f32)
            # small inputs first so the cheap losses overlap the feature DMA;
            # features staged so block 0 (columns < 512) starts early
            nc.sync.dma_start(out=tok[:], in_=d_tok[:])
            nc.sync.dma_start(out=tri[:], in_=d_tri[:])
            fv_d = d_feat[:].rearrange("k (s l) -> k s l", s=4)
            fv_s = feat[:].rearrange("k (s l) -> k s l", s=4)
            cuts = [0, min(512, L), min(1536, L), L]
            for a, b in zip(cuts[:-1], cuts[1:]):
                if b > a:
                    nc.sync.dma_start(out=fv_s[:, :, a:b], in_=fv_d[:, :, a:b])

            fA12 = feat[:, 0 * L:1 * L]
            fB12 = feat[:, 1 * L:2 * L]
            fA34 = feat[0:KQ34, 2 * L:3 * L]
            fB34 = feat[0:KQ34, 3 * L:4 * L]
            ppn = tok[:, 0:32]
            tpn = tok[:, 32:64]
            opr = tok[:, 64:96]
            ton = tok[:, 96:128]
            cp4 = tok[:, 128:192]
            ohv = tok[:, 192:256]
            vf = tok[:, 256:272]

            cols = singles.tile([128, NCOLS], f32)
            junk = singles.tile([128, 512], f32)
            dpt = singles.tile([128, 32], f32)
            e4 = singles.tile([128, 64], f32)
            gs = singles.tile([128, 16], f32)
            lg = singles.tile([128, 16], f32)

            nc.vector.memset(cols, 0.0)

            # ---- cheap losses (ACT: Exp then Ln, before the sigmoid set) ---
            # col0: sum (pp-tp)^2 * nn   (host scales by 0.25/RET^2)
            nc.vector.tensor_tensor(out=dpt[:], in0=ppn, in1=tpn,
                                    op=ALU.subtract)
            nc.vector.tensor_tensor(out=junk[:, :32], in0=dpt[:], in1=dpt[:],
                                    op=ALU.mult)
            nc.vector.tensor_reduce(out=cols[:, 0:1], in_=junk[:, :32],
                                    axis=AX.X, op=ALU.add)
            # col1: sum (op . to) * nn
            nc.vector.tensor_tensor(out=junk[:, :32], in0=opr, in1=ton,
                                    op=ALU.mult)
            nc.vector.tensor_reduce(out=cols[:, 1:2], in_=junk[:, :32],
                                    axis=AX.X, op=ALU.add)
            # col2: sum lse*vf ; col3: sum x_sel*vf
            nc.scalar.activation(out=e4[:], in_=cp4, func=ACT.Exp)
            nc.vector.tensor_reduce(
                out=gs[:], in_=e4[:].rearrange("p (t c) -> p t c", c=4),
                axis=AX.X, op=ALU.add)
            nc.scalar.activation(out=lg[:], in_=gs[:], func=ACT.Ln)
            nc.vector.tensor_tensor(out=junk[:, :16], in0=lg[:], in1=vf,
                                    op=ALU.mult)
            nc.vector.tensor_reduce(out=cols[:, 2:3], in_=junk[:, :16],
                                    axis=AX.X, op=ALU.add)
            nc.vector.tensor_tensor(out=junk[:, :64], in0=cp4, in1=ohv,
                                    op=ALU.mult)
            nc.vector.tensor_reduce(out=cols[:, 3:4], in_=junk[:, :64],
                                    axis=AX.X, op=ALU.add)

            # ---- intersection grid ----
            sdt = bf16 if SIG_BF16 else f32

            for blk, sb in enumerate(sched):
                # per chunk k: q12 -> bank 2k, q34 -> bank 2k+1
                q = psum.tile([128, 2048], f32, tag="q")
                for k, (i0, j0, N, first) in enumerate(sb):
                    nc.tensor.matmul(q[:, 1024 * k:1024 * k + N],
                                     fA12[:, i0:i0 + 128],
                                     fB12[:, j0:j0 + N],
                                     start=True, stop=True)
                    nc.tensor.matmul(q[:, 1024 * k + 512:1024 * k + 512 + N],
                                     fA34[:, i0:i0 + 128],
                                     fB34[:, j0:j0 + N],
                                     start=True, stop=True)
                s12 = sig.tile([128, 2048], sdt, tag="s12")
                if len(sb) == 2 and ACT_4BANK:  # two full 512-chunk pairs
                    nc.scalar.activation(out=s12[:], in_=q[:],
                                         func=ACT.Sigmoid, scale=-0.01)
                elif len(sb) == 2:
                    nc.scalar.activation(out=s12[:, 0:1024], in_=q[:, 0:1024],
                                         func=ACT.Sigmoid, scale=-0.01)
                    nc.scalar.activation(out=s12[:, 1024:2048],
                                         in_=q[:, 1024:2048],
                                         func=ACT.Sigmoid, scale=-0.01)
                else:
                    N = sb[0][2]
                    qv = q[:, 0:1024].rearrange(
                        "p (two n) -> p two n", two=2)[:, :, :N]
                    sv = s12[:, 0:1024].rearrange(
                        "p (two n) -> p two n", two=2)[:, :, :N]
                    nc.scalar.activation(out=sv, in_=qv,
                                         func=ACT.Sigmoid, scale=-0.01)
                # triangular mask on the s1 quadrant
                for k, (i0, j0, N, first) in enumerate(sb):
                    if first:
                        W = min(W_DIAG, N)
                        off = 1024 * k
                        if INPLACE_MASK:
                            nc.vector.tensor_tensor(
                                out=s12[:, off:off + W],
                                in0=s12[:, off:off + W],
                                in1=tri[:, :W], op=ALU.mult)
                        else:
                            s1m = sig.tile([128, W_DIAG], sdt, tag="s1m")
                            nc.vector.tensor_tensor(
                                out=s1m[:, :W], in0=s12[:, off:off + W],
                                in1=tri[:, :W], op=ALU.mult)
                            nc.vector.tensor_copy(
                                out=s12[:, off:off + W], in_=s1m[:, :W])
                # s1*s2 product, then row-sum into this superblock's column
                prod = sig.tile([128, 1024], sdt, tag="prod")
                prod_eng = nc.gpsimd if GP_PROD else nc.vector
                if len(sb) == 2:
                    v = s12[:].rearrange("p (k n) -> p k n", n=512)
                    prod_eng.tensor_tensor(
                        out=prod[:].rearrange("p (k n) -> p k n", n=512),
                        in0=v[:, 0::2, :], in1=v[:, 1::2, :], op=ALU.mult)
                    red_in = prod[:]
                else:
                    N = sb[0][2]
                    prod_eng.tensor_tensor(out=prod[:, :N],
                                           in0=s12[:, 0:N],
                                           in1=s12[:, 512:512 + N],
                                           op=ALU.mult)
                    red_in = prod[:, :N]
                c = ISECT_COL0 + blk
                nc.vector.tensor_reduce(out=cols[:, c:c + 1], in_=red_in,
                                        axis=AX.X, op=ALU.add)

            nc.sync.dma_start(out=d_out[:], in_=cols[:])

    _split_multi_waits(nc)
    return nc


def kernel(point_pred, orient_pred, class_pred, target_seq, padding_mask):
    pp = np.ascontiguousarray(np.asarray(point_pred, dtype=np.float32))
    op = np.ascontiguousarray(np.asarray(orient_pred, dtype=np.float32))
    cp = np.ascontiguousarray(np.asarray(class_pred, dtype=np.float32))
    ts = np.ascontiguousarray(np.asarray(target_seq, dtype=np.float32))
    pm = np.ascontiguousarray(np.asarray(padding_mask)).astype(bool)

    tc_cls, tp, to, valid, nn, per_core, n_segs = _host_prep(pp, op, cp, ts, pm)

    # grid bounds: i-bands padded to 128; j truncated at max n_seg (mult of 4)
    nsmax = max(max(n_segs), 1)
    L = max(128, -(-nsmax // 128) * 128)
    L = min(L, -(-M // 128) * 128)  # ceil(M/128)*128 = 2048 max
    Jmax = min(-(-nsmax // 4) * 4, L)

    key = (L, Jmax, SIG_BF16, ACT_4BANK, INPLACE_MASK, GP_PROD)
    if key not in _CACHE:
        _CACHE[key] = _build_program(L, Jmax)
    nc = _CACHE[key]

    # triangular mask for the leading chunk of each band: keep j-i >= 2
    ii = np.arange(128)[:, None]
    jj = np.arange(W_DIAG)[None, :]
    tri = (jj >= ii + 2).astype(np.float32)
    if SIG_BF16:
        import ml_dtypes
        tri = tri.astype(ml_dtypes.bfloat16)

    import ml_dtypes
    bfdt = ml_dtypes.bfloat16
    eye = np.eye(NUM_CLASSES, dtype=np.float32)
    in_maps = []
    for b in range(B):
        pc = per_core[b]
        featpk = np.zeros((KQ12, 4 * L), bfdt)
        w = min(M, L)
        featpk[:KQ12, 0 * L:0 * L + w] = pc["A12"][:, :w]
        featpk[:KQ12, 1 * L:1 * L + w] = pc["B12"][:, :w]
        featpk[:KQ34, 2 * L:2 * L + w] = pc["A34"][:, :w]
        featpk[:KQ34, 3 * L:3 * L + w] = pc["B34"][:, :w]
        if L > M:
            # mask the padded tail: A12 row6=BIG*inv_i / row7=ones pattern.
            # In the split stacking, group g occupies rows 8g..8g+7 with
            # A-pattern [h,m,h,l,h,m] / B-pattern [h,h,m,h,l,m]; setting the
            # hi part of group 0 is enough (BIG in bf16 is still huge).
            big_bf = bfdt(BIG)
            featpk[6, 0 * L + M:1 * L] = big_bf   # A12 g0(hi) row6 = BIG
            featpk[7, 0 * L + M:1 * L] = bfdt(1.0)
            featpk[6, 1 * L + M:2 * L] = bfdt(1.0)
            featpk[7, 1 * L + M:2 * L] = big_bf

        nf = nn[b].astype(np.float32)[:, None]
        vfb = valid[b].astype(np.float32)
        tokpk = np.concatenate([
            (pp[b] * nf).reshape(128, 32),
            (tp[b] * nf).reshape(128, 32),
            op[b].reshape(128, 32),
            (to[b] * nf).reshape(128, 32),
            cp[b].reshape(128, 64),
            (eye[tc_cls[b]] * vfb[:, None]).reshape(128, 64),
            vfb.reshape(128, 16),
        ], axis=1).astype(np.float32)
        in_maps.append({
            "feat": np.ascontiguousarray(featpk),
            "tri": tri,
            "tok": np.ascontiguousarray(tokpk),
        })

    from concourse.bass_utils import run_bass_kernel_spmd
    global LAST_RESULTS
    kw = dict(TRACE_KWARGS) if TRACE_KWARGS else {}
    res = run_bass_kernel_spmd(nc, in_maps, core_ids=list(range(NCORES)), **kw)
    LAST_RESULTS = res
    parts = [r["partials"] for r in res.results]  # each [128, NCOLS] f32

    f32 = np.float32
    pt_raw = f32(0); cos_sum = f32(0); lse_sum = f32(0); sel_sum = f32(0)
    isect_sum = f32(0)
    for b in range(B):
        p = parts[b].astype(np.float32)
        pt_raw += p[:, 0].sum(dtype=np.float32)
        cos_sum += p[:, 1].sum(dtype=np.float32)
        lse_sum += p[:, 2].sum(dtype=np.float32)
        sel_sum += p[:, 3].sum(dtype=np.float32)
        isect_sum += p[:, ISECT_COL0:].sum(dtype=np.float32)

    # wrap-pair exclusion + pair count (host, exact)
    wrap_sum = np.float64(0.0)
    cnt_total = 0
    for b in range(B):
        pc = per_core[b]
        n, n_seg = pc["n"], pc["n_seg"]
        if n < 4:
            continue
        cnt_total += (n_seg - 1) * (n_seg - 2) // 2 - 1
        jw = n_seg - 1
        q12w = np.float32(
            np.dot(pc["A12"][:, 0].astype(np.float32),
                   pc["B12"][:, jw].astype(np.float32)))
        q34w = np.float32(
            np.dot(pc["A34"][:, 0].astype(np.float32),
                   pc["B34"][:, jw].astype(np.float32)))
        with np.errstate(over="ignore"):
            sw = (1.0 / (1.0 + np.exp(np.float64(0.01) * q12w)) *
                  1.0 / (1.0 + np.exp(np.float64(0.01) * q34w)))
        wrap_sum += sw

    valid_cnt = f32(valid.sum())
    nn_cnt = f32(nn.sum())
    vden = max(valid_cnt, f32(1.0))
    nden = max(nn_cnt, f32(1.0))

    pt_loss = f32(pt_raw * f32(0.25 / (RETINA * RETINA)) / nden)
    orient_loss = f32((nn_cnt - cos_sum) / nden)
    cls_loss = f32((lse_sum - sel_sum) / vden)
    if cnt_total > 0:
        isect_loss = f32((np.float64(isect_sum) - wrap_sum) / cnt_total)
    else:
        isect_loss = f32(0.0)
    total = f32(pt_loss + f32(0.5) * orient_loss + cls_loss
                + f32(0.1) * isect_loss)
    return (total, pt_loss, orient_loss, cls_loss, isect_loss)
